# revision 1
# baseline (speedup 1.0000x reference)
"""Trainium2 Bass kernel for BiLSTM-CRF loss (nn_CWS_10952166605290).

Distribution: data-parallel over batch, 8 cores x 8 sequences.

Device pipeline per core (validated against the jax reference in numpy):
- Embedding gather via indirect DMA from a bf16 table copy; PE-transpose puts
  E on partitions for the input projection.
- Input projection (W_ih + fused biases) lands in PSUM 16 scan-steps at a
  time; per-step recurrent gate matmuls accumulate onto it.
- sigma-only LSTM cell: tanh(x) = 2*sigmoid(2x)-1 using host-prescaled
  weights; h stored as h_hat = h/2 (consumers pre-doubled). Per step:
  2 sigmoid ACTs, 4 vector ops, 8 tiny matmuls.
- Forward scan unmasked; backward scan globally time-reversed with per-column
  (h0,c0) blend-injection where each sequence's valid region begins
  (per-core masks keep the SPMD program uniform).
- Emissions = one matmul pair per sequence into PSUM (backward read with a
  negative-stride rhs); exp(E + b_out - kappa) on ScalarE; numerator and
  final-alpha selection via masked accumulate ops.
- CRF forward algorithm in the exp domain: p_t = (expA.T @ p_{t-1}) * expE_t
  (tiny matmul + vector multiply per step) with periodic renormalization.
- Host reconstructs the scalar loss from per-sequence pieces.
"""
import numpy as np
import ml_dtypes

import concourse.bass as bass
import concourse.mybir as mybir
import concourse.tile as tile
from concourse.masks import make_identity

BF16 = ml_dtypes.bfloat16
F32 = mybir.dt.float32
BF = mybir.dt.bfloat16
I32 = mybir.dt.int32
AF = mybir.ActivationFunctionType
AL = mybir.AluOpType

NCORES = 8
B, S, V, E, Hh, T = 64, 512, 100001, 256, 128, 4
BPC = B // NCORES
KAPPA = float(np.log(T))
CHUNK = 16


def _default_renorms(s_len):
    return tuple(r for r in (127, 255, 383) if r < s_len - 1) if s_len > 128 else \
        (tuple((s_len // 2,)) if s_len >= 32 else tuple())




def _split_multi_waits(nc):
    """Walrus in this toolchain accepts at most ONE sync wait per instruction.
    Split extra waits into single-wait NoOps queued just before, on the same
    engine."""
    k = 0
    for fn in nc.m.functions:
        for blk in fn.blocks:
            insts = blk.instructions
            out = []
            changed = False
            for inst in insts:
                si = inst.sync_info
                if si is not None and si.on_wait and len(si.on_wait) > 1:
                    waits = list(si.on_wait)
                    for w in waits[:-1]:
                        nop = mybir.InstNoOp(name=f"wsplit_{k}")
                        k += 1
                        nop.engine = inst.engine
                        nop.sync_info = mybir.SyncInfo(on_wait=[w], on_update=[])
                        out.append(nop)
                    si.on_wait = [waits[-1]]
                    inst.sync_info = si
                    changed = True
                out.append(inst)
            if changed:
                blk.instructions = out
    return k


def _blob_layout(s_len, bpc, ngrp, tpg, nslot):
    """Per-partition byte offsets for the packed constants blob."""
    sects = {}
    off = 0
    def add(name, nbytes):
        nonlocal off
        sects[name] = off
        off += (nbytes + 3) // 4 * 4
    add("whh", 2 * 4 * Hh * 2)            # bf16 [2,4,Hh]
    add("wih", 2 * 4 * 2 * Hh * 2)        # bf16 [2,4,2,Hh]
    add("bias", 2 * 4 * Hh * 4)           # f32 row0 [8,Hh]
    add("h0c0", 2 * 2 * bpc * 4)          # f32 [2,2,bpc]
    add("wout", 2 * T * 2)                # bf16 [2,T]
    add("boutk", 4)                       # f32 rows0-3 [1]
    add("expa", T * 4)                    # f32 rows0-3 [T]
    add("expstart", 4)                    # f32 rows0-3 [1]
    add("idx", 2 * ngrp * tpg * 4)        # i32 [2,ngrp,tpg]
    add("ohm", 2 * bpc * s_len * 4)       # f32 rows0-3 [2,bpc*s_len]
    add("keep", max(1, nslot) * bpc * 4)  # f32 [nslot,bpc] replicated
    add("hcinj", max(1, nslot) * 2 * bpc * 4)  # f32 [nslot,2,bpc]
    return off, sects


def build_program(s_len=S, bpc=BPC, renorms=None, inj_steps=(), n_vocab=V, debug=False, split_waits=True):
    """Build the SPMD bass program (one program shared by all cores)."""
    if renorms is None:
        renorms = _default_renorms(s_len)
    renorms = tuple(renorms)
    inj_steps = tuple(inj_steps)
    nslot = len(inj_steps)
    nc = bass.Bass()

    nch = s_len // CHUNK                 # scan chunks
    tpg = 4                              # token-rows per partition per gather group
    gtok = 128 * tpg                     # tokens per gather group
    ngrp = (s_len * bpc + gtok - 1) // gtok
    nout = bpc + bpc + len(renorms) * bpc

    table = nc.dram_tensor("table", [n_vocab, E], BF, kind="ExternalInput")
    off, sects = _blob_layout(s_len, bpc, ngrp, tpg, nslot)
    blob = nc.dram_tensor("blob", [128, off], mybir.dt.uint8, kind="ExternalInput")
    devout = nc.dram_tensor("devout", [T, nout], F32, kind="ExternalOutput")
    if debug:
        dbg_h = nc.dram_tensor("dbg_h", [128, 2, bpc, s_len + 1], BF,
                               kind="ExternalOutput")
        dbg_e = nc.dram_tensor("dbg_e", [T, bpc, s_len], F32, kind="ExternalOutput")
        dbg_p = nc.dram_tensor("dbg_p", [T, bpc, s_len], F32, kind="ExternalOutput")
        dbg_x = nc.dram_tensor("dbg_x", [128, 2, nch, 2, CHUNK * bpc], BF,
                               kind="ExternalOutput")

    with tile.TileContext(nc) as tc:
        with (
            tc.tile_pool(name="const", bufs=1) as constp,
            tc.tile_pool(name="hist", bufs=1) as histp,
            tc.tile_pool(name="emb", bufs=1) as embp,
            tc.tile_pool(name="gath", bufs=16) as gathp,
            tc.tile_pool(name="state", bufs=2) as statep,
            tc.tile_pool(name="work", bufs=2) as workp,
            tc.tile_pool(name="post", bufs=1) as postp,
        ):
            # ---------------- constants (single packed DMA) ----------------
            blob_sb = constp.tile([128, off], mybir.dt.uint8, tag="blob")
            nc.sync.dma_start(out=blob_sb, in_=blob[:])

            def view(name, nbytes, dt, rows=128):
                o = sects[name]
                return blob_sb[0:rows, o:o + nbytes].bitcast(dt)

            whh_sb = view("whh", 2 * 4 * Hh * 2, BF).rearrange(
                "k (d g m) -> k d g m", d=2, g=4)
            wih_sb = view("wih", 2 * 4 * 2 * Hh * 2, BF).rearrange(
                "k (d g j m) -> k d g j m", d=2, g=4, j=2)
            bias_sb = view("bias", 2 * 4 * Hh * 4, F32, rows=1).rearrange(
                "k (dg m) -> k dg m", dg=8)
            h0c0_sb = view("h0c0", 2 * 2 * bpc * 4, F32).rearrange(
                "k (a d b) -> k a d b", a=2, d=2)
            wout_sb = view("wout", 2 * T * 2, BF).rearrange(
                "k (d t) -> k d t", d=2)
            boutk_sb = view("boutk", 4, F32, rows=T)
            expa_sb = view("expa", T * 4, F32, rows=T)
            expstart_sb = view("expstart", 4, F32, rows=T)
            idx_sb = view("idx", 2 * ngrp * tpg * 4, I32).rearrange(
                "k (d n j) -> k d n j", d=2, n=ngrp)
            ohm_sb = view("ohm", 2 * bpc * s_len * 4, F32, rows=T).rearrange(
                "k (a c) -> k a c", a=2)
            if nslot:
                keep_sb = view("keep", nslot * bpc * 4, F32).rearrange(
                    "k (s b) -> k s b", s=nslot)
                hcinj_sb = view("hcinj", nslot * 2 * bpc * 4, F32).rearrange(
                    "k (s a b) -> k s a b", s=nslot, a=2)

            ones_sb = constp.tile([128, CHUNK * bpc], F32, tag="ones")
            nc.vector.memset(ones_sb, 1.0)
            h0_bf = constp.tile([128, 2, bpc], BF)
            nc.vector.tensor_copy(out=h0_bf, in_=h0c0_sb[:, 0, :, :])
            ident = constp.tile([128, 128], BF, tag="ident")
            make_identity(nc, ident)

            # ---------------- h history ----------------
            hhist = histp.tile([128, 2, bpc, s_len + 1], BF)
            nc.vector.tensor_copy(out=hhist[:, :, :, 0], in_=h0_bf)

            # embeddings, transposed, per chunk: [128, dir, chunk, Ehalf, CHUNK*bpc]
            embT = embp.tile([128, 2, nch, 2, CHUNK * bpc], BF)

            with tc.tile_pool(name="scanps", bufs=2, space="PSUM") as scanps:
                # ---------------- gather + transpose ----------------
                for d in range(2):
                    for g in range(ngrp):
                        for j in range(tpg):
                            ch = g * tpg + j
                            if ch >= nch:
                                continue
                            gt = gathp.tile([128, E], BF, tag="gath")
                            nc.gpsimd.indirect_dma_start(
                                out=gt, out_offset=None, in_=table[:],
                                in_offset=bass.IndirectOffsetOnAxis(
                                    ap=idx_sb[:, d, g, j:j + 1], axis=0))
                            for hf in range(2):
                                tp = scanps.tile([128, 128], BF, tag="tp")
                                nc.tensor.transpose(
                                    out=tp,
                                    in_=gt[:, hf * 128:(hf + 1) * 128],
                                    identity=ident)
                                nc.scalar.copy(
                                    out=embT[:, d, ch, hf, :],
                                    in_=tp[:, :CHUNK * bpc])

                # ---------------- LSTM scans ----------------
                c_prev = statep.tile([128, 2, bpc], F32, tag="c")
                nc.vector.tensor_copy(out=c_prev, in_=h0c0_sb[:, 1, :, :])
                slot_i = 0
                gp = None
                for t in range(s_len):
                    tt = t % CHUNK
                    ch = t // CHUNK
                    if tt == 0:
                        gp = scanps.tile([128, 2, 4, CHUNK, bpc], F32, tag="gp")
                        for d in range(2):
                            for gq in range(4):
                                for j in range(2):
                                    nc.tensor.matmul(
                                        out=gp[:, d, gq, :, :],
                                        lhsT=wih_sb[:, d, gq, j, :],
                                        rhs=embT[:, d, ch, j, :].rearrange(
                                            "k (s b) -> k s b", b=bpc),
                                        start=(gq == 0 and j == 0),
                                        stop=False, skip_group_check=True)
                                nc.tensor.matmul(
                                    out=gp[:, d, gq, :, :],
                                    lhsT=bias_sb[0:1, d * 4 + gq, :],
                                    rhs=ones_sb[0:1, :].rearrange(
                                        "k (s b) -> k s b", b=bpc),
                                    start=False, stop=False,
                                    skip_group_check=True)
                    for d in range(2):
                        rhs = hhist[:, d, :, t]
                        for gq in range(4):
                            nc.tensor.matmul(
                                out=gp[:, d, gq, tt, :],
                                lhsT=whh_sb[:, d, gq, :],
                                rhs=rhs,
                                start=False,
                                stop=(tt == CHUNK - 1 and gq == 3),
                                skip_group_check=True)
                    g_sb = workp.tile([128, 2, 4, bpc], F32, tag="g")
                    nc.scalar.activation(out=g_sb, in_=gp[:, :, :, tt, :],
                                         func=AF.Sigmoid)
                    a_t = workp.tile([128, 2, bpc], F32, tag="a")
                    nc.vector.tensor_tensor(out=a_t, in0=g_sb[:, :, 1, :],
                                            in1=c_prev, op=AL.mult)
                    u_t = workp.tile([128, 2, bpc], F32, tag="u")
                    nc.vector.scalar_tensor_tensor(
                        out=u_t, in0=g_sb[:, :, 2, :], scalar=0.5,
                        in1=g_sb[:, :, 0, :], op0=AL.subtract, op1=AL.mult)
                    c_new = statep.tile([128, 2, bpc], F32, tag="c")
                    nc.vector.scalar_tensor_tensor(
                        out=c_new, in0=u_t, scalar=2.0, in1=a_t,
                        op0=AL.mult, op1=AL.add)
                    sc_t = workp.tile([128, 2, bpc], F32, tag="sc")
                    nc.scalar.activation(out=sc_t, in_=c_new,
                                         func=AF.Sigmoid, scale=2.0)
                    nc.vector.scalar_tensor_tensor(
                        out=hhist[:, :, :, t + 1], in0=sc_t, scalar=0.5,
                        in1=g_sb[:, :, 3, :], op0=AL.subtract, op1=AL.mult)
                    c_prev = c_new

                    # backward-state blend injections for step t+1
                    if slot_i < nslot and inj_steps[slot_i] == t + 1:
                        hcol = hhist[:, 1, :, t + 1]
                        tmp = workp.tile([128, 2, bpc], F32, tag="tmp")
                        nc.vector.tensor_tensor(
                            out=tmp[:, 0, :], in0=hcol,
                            in1=keep_sb[:, slot_i, :], op=AL.mult)
                        nc.vector.tensor_tensor(
                            out=hcol, in0=tmp[:, 0, :],
                            in1=hcinj_sb[:, slot_i, 0, :], op=AL.add)
                        nc.vector.tensor_tensor(
                            out=tmp[:, 1, :], in0=c_prev[:, 1, :],
                            in1=keep_sb[:, slot_i, :], op=AL.mult)
                        nc.vector.tensor_tensor(
                            out=c_prev[:, 1, :], in0=tmp[:, 1, :],
                            in1=hcinj_sb[:, slot_i, 1, :], op=AL.add)
                        slot_i += 1

            # ---------------- emissions + exp + numerator ----------------
            expe = postp.tile([T, bpc, s_len], F32, tag="expe")
            dout = postp.tile([T, nout], F32, tag="dout")
            scratch = postp.tile([T, s_len], F32, tag="scr")
            with tc.tile_pool(name="eps", bufs=1, space="PSUM") as epsp:
                epsum = epsp.tile([T, bpc, s_len], F32)
                for b in range(bpc):
                    nc.tensor.matmul(
                        out=epsum[:, b, :], lhsT=wout_sb[:, 0, :],
                        rhs=hhist[:, 0, b, 1:s_len + 1],
                        start=True, stop=False)
                    hb = hhist[:, 1, b, 0]
                    rev = bass.AP(
                        tensor=hb.tensor,
                        offset=hb.offset + s_len,
                        ap=[hb.ap[0], [-1, s_len]])
                    nc.tensor.matmul(
                        out=epsum[:, b, :], lhsT=wout_sb[:, 1, :],
                        rhs=rev, start=False, stop=True)
                    nc.vector.scalar_tensor_tensor(
                        out=scratch, in0=epsum[:, b, :], scalar=0.0,
                        in1=ohm_sb[:, 0, b * s_len:(b + 1) * s_len],
                        op0=AL.add, op1=AL.mult,
                        accum_out=dout[:, b:b + 1])
                    nc.scalar.activation(
                        out=expe[:, b, :], in_=epsum[:, b, :],
                        func=AF.Exp, bias=boutk_sb[:, :], scale=1.0)

            # ---------------- CRF scan ----------------
            phist = postp.tile([T, bpc, s_len], F32, tag="phist")
            with tc.tile_pool(name="crfps", bufs=2, space="PSUM") as crfps:
                nc.vector.tensor_tensor(
                    out=phist[:, :, 0], in0=expe[:, :, 0],
                    in1=expstart_sb.broadcast_to([T, bpc]), op=AL.mult)
                prev = phist[:, :, 0]
                ri = 0
                for t in range(1, s_len):
                    q = crfps.tile([T, bpc], F32, tag="q")
                    nc.tensor.matmul(out=q, lhsT=expa_sb, rhs=prev,
                                     start=True, stop=True)
                    nc.vector.tensor_tensor(
                        out=phist[:, :, t], in0=q, in1=expe[:, :, t],
                        op=AL.mult)
                    prev = phist[:, :, t]
                    if ri < len(renorms) and renorms[ri] == t:
                        sp = crfps.tile([1, bpc], F32, tag="sp")
                        nc.tensor.matmul(out=sp, lhsT=ones_sb[0:T, 0:1],
                                         rhs=prev, start=True, stop=True)
                        rs = workp.tile([1, bpc], F32, tag="rs")
                        nc.vector.reciprocal(out=rs, in_=sp)
                        rb = crfps.tile([T, bpc], F32, tag="rb")
                        nc.tensor.matmul(out=rb, lhsT=ones_sb[0:1, 0:T],
                                         rhs=rs, start=True, stop=True)
                        pren = workp.tile([T, bpc], F32, tag="pren")
                        nc.vector.tensor_tensor(out=pren, in0=prev, in1=rb,
                                                op=AL.mult)
                        prev = pren
                        ri += 1

                # final-alpha selection + renorm snapshots
                for b in range(bpc):
                    nc.vector.scalar_tensor_tensor(
                        out=scratch, in0=phist[:, b, :], scalar=0.0,
                        in1=ohm_sb[:, 1, b * s_len:(b + 1) * s_len],
                        op0=AL.add, op1=AL.mult,
                        accum_out=dout[:, bpc + b:bpc + b + 1])
                for r, tr in enumerate(renorms):
                    nc.vector.tensor_copy(
                        out=dout[:, 2 * bpc + r * bpc:2 * bpc + (r + 1) * bpc],
                        in_=phist[:, :, tr])

            nc.sync.dma_start(out=devout[:], in_=dout)
            if debug:
                nc.sync.dma_start(out=dbg_h[:], in_=hhist)
                nc.sync.dma_start(out=dbg_e[:], in_=expe)
                nc.sync.dma_start(out=dbg_p[:], in_=phist)
                nc.sync.dma_start(out=dbg_x[:], in_=embT)

    nsplit = _split_multi_waits(nc) if split_waits else 0
    return nc, dict(s_len=s_len, bpc=bpc, renorms=renorms, inj_steps=inj_steps,
                    ngrp=ngrp, tpg=tpg, nch=nch, nout=nout, n_vocab=n_vocab,
                    nsplit=nsplit)


def host_prepare(inputs, s_len=S, bpc=BPC, n_cores=NCORES, n_vocab=V):
    """Host-side preprocessing: per-core input maps + per-sequence constants."""
    I = {k: np.asarray(v) for k, v in inputs.items()}
    sent = I['sentence'].astype(np.int64)
    tags = I['tags'].astype(np.int64)
    mask = I['mask'].astype(bool)
    length = I['length'].astype(np.int64)
    nb = sent.shape[0]

    L = mask.sum(axis=1).astype(np.int64)
    if not np.all(mask == (np.arange(s_len)[None, :] < L[:, None])):
        raise ValueError("mask is not a prefix mask; kernel requires prefix masks")
    assert np.all(L >= 1)

    # ---- prescaled weights ----
    def prep(Wih, Whh, bih, bhh):
        Wih = Wih.astype(np.float64).copy()
        Whh = Whh.astype(np.float64).copy() * 2.0
        bb = (bih.astype(np.float64) + bhh.astype(np.float64)).copy()
        Wih[2 * Hh:3 * Hh] *= 2.0
        Whh[2 * Hh:3 * Hh] *= 2.0
        bb[2 * Hh:3 * Hh] *= 2.0
        return Wih, Whh, bb
    WihF, WhhF, bF = prep(I['W_ih_f'], I['W_hh_f'], I['b_ih_f'], I['b_hh_f'])
    WihB, WhhB, bB = prep(I['W_ih_b'], I['W_hh_b'], I['b_ih_b'], I['b_hh_b'])

    whh_arr = np.zeros((2, 4, Hh, Hh), BF16)
    wih_arr = np.zeros((2, 4, 2, 128, Hh), BF16)
    bias_arr = np.zeros((2 * 4, Hh), np.float32)
    for d, (Wih, Whh, bb) in enumerate(((WihF, WhhF, bF), (WihB, WhhB, bB))):
        for g in range(4):
            whh_arr[d, g] = Whh[g * Hh:(g + 1) * Hh, :].T.astype(BF16)
            for j in range(2):
                wih_arr[d, g, j] = Wih[g * Hh:(g + 1) * Hh,
                                       j * 128:(j + 1) * 128].T.astype(BF16)
            bias_arr[d * 4 + g] = bb[g * Hh:(g + 1) * Hh].astype(np.float32)

    wout_arr = np.zeros((2, 128, T), BF16)
    Wo = I['W_out'].astype(np.float64) * 2.0
    wout_arr[0] = Wo[:, :Hh].T.astype(BF16)
    wout_arr[1] = Wo[:, Hh:].T.astype(BF16)
    boutk_arr = (I['b_out'].astype(np.float64) - KAPPA).astype(np.float32).reshape(T, 1)
    expa_arr = np.exp(I['trans'].astype(np.float64)).astype(np.float32)
    expstart_arr = np.exp(I['start_trans'].astype(np.float64)).astype(np.float32).reshape(T, 1)
    expend = np.exp(I['end_trans'].astype(np.float64))

    table_bf = I['embed'].astype(BF16)
    if table_bf.shape[0] < n_vocab:
        pad = np.zeros((n_vocab - table_bf.shape[0], E), BF16)
        table_bf = np.concatenate([table_bf, pad], 0)

    h0hat = I['h0'].astype(np.float64) / 2.0         # [2, B, Hh]
    c0 = I['c0'].astype(np.float64)

    # ---- injection union schedule ----
    inj_of_b = {}
    for bg in range(nb):
        if L[bg] < s_len:
            inj_of_b[bg] = s_len - int(L[bg])        # blend before this step
    inj_steps = tuple(sorted(set(inj_of_b.values())))
    nslot = len(inj_steps)
    slot_of = {s: i for i, s in enumerate(inj_steps)}

    # ---- host constants per sequence ----
    maskf = mask.astype(np.float64)
    mtilde = maskf.copy()
    mtilde[:, 0] = 1.0
    start = I['start_trans'].astype(np.float64)
    end = I['end_trans'].astype(np.float64)
    trans = I['trans'].astype(np.float64)
    bshift = (I['b_out'].astype(np.float64) - KAPPA)
    const = start[tags[:, 0]].copy()
    const += (trans[tags[:, :-1], tags[:, 1:]] * maskf[:, 1:]).sum(axis=1)
    const += end[tags[np.arange(nb), length - 1]]
    const += (bshift[tags] * mtilde).sum(axis=1)     # device numerator excludes bias

    renorms = _default_renorms(s_len)

    in_maps = []
    percore = []
    nch = s_len // CHUNK
    tpg = 4
    gtok = 128 * tpg
    ngrp = (s_len * bpc + gtok - 1) // gtok
    for ci in range(n_cores):
        bsl = slice(ci * bpc, (ci + 1) * bpc)
        bg0 = ci * bpc
        # gather indices
        idx_arr = np.zeros((2, ngrp, 128, tpg), np.int32)
        for d in range(2):
            for g in range(ngrp):
                for j in range(tpg):
                    ch = g * tpg + j
                    if ch >= nch:
                        continue
                    for p in range(128):
                        tt = p // bpc
                        b = p % bpc
                        s_step = ch * CHUNK + tt
                        t_true = s_step if d == 0 else (s_len - 1 - s_step)
                        idx_arr[d, g, p, j] = sent[bg0 + b, t_true]
        # ohm planes
        ohm_arr = np.zeros((T, 2, bpc * s_len), np.float32)
        for b in range(bpc):
            bg = bg0 + b
            for t in range(s_len):
                col = b * s_len + t
                if mtilde[bg, t] > 0:
                    ohm_arr[tags[bg, t], 0, col] = 1.0
                if t == L[bg] - 1:
                    ohm_arr[:, 1, col] = 1.0
        # h0c0 + injections
        h0c0_arr = np.zeros((128, 2, 2, bpc), np.float32)
        h0c0_arr[:, 0, :, :] = h0hat[:, bsl, :].transpose(2, 0, 1)
        h0c0_arr[:, 1, :, :] = c0[:, bsl, :].transpose(2, 0, 1)
        keep_arr = np.ones((max(1, nslot), bpc), np.float32)
        hcinj_arr = np.zeros((max(1, nslot), 2, 128, bpc), np.float32)
        for b in range(bpc):
            bg = bg0 + b
            if bg in inj_of_b:
                sl = slot_of[inj_of_b[bg]]
                keep_arr[sl, b] = 0.0
                hcinj_arr[sl, 0, :, b] = h0hat[1, bg, :]
                hcinj_arr[sl, 1, :, b] = c0[1, bg, :]
        off, sects = _blob_layout(s_len, bpc, ngrp, tpg, nslot)
        blob = np.zeros((128, off), np.uint8)

        def put(name, rows, arr):
            bts = np.ascontiguousarray(arr).view(np.uint8).reshape(rows, -1)
            blob[:rows, sects[name]:sects[name] + bts.shape[1]] = bts

        put("whh", 128, whh_arr.transpose(2, 0, 1, 3))
        put("wih", 128, wih_arr.transpose(3, 0, 1, 2, 4))
        put("bias", 1, bias_arr[None])
        put("h0c0", 128, h0c0_arr)
        put("wout", 128, wout_arr.transpose(1, 0, 2))
        put("boutk", T, boutk_arr)
        put("expa", T, expa_arr)
        put("expstart", T, expstart_arr)
        put("idx", 128, idx_arr.transpose(2, 0, 1, 3))
        put("ohm", T, ohm_arr)
        put("keep", 128, np.broadcast_to(keep_arr[None], (128,) + keep_arr.shape))
        put("hcinj", 128, hcinj_arr.transpose(2, 0, 1, 3))
        in_maps.append({
            "table": np.ascontiguousarray(table_bf),
            "blob": blob,
        })
        percore.append(dict(bg0=bg0))
    hostctx = dict(const=const, expend=expend, L=L, renorms=renorms,
                   inj_steps=inj_steps, nb=nb)
    return in_maps, hostctx


def host_finish(results, hostctx, s_len=S, bpc=BPC):
    """Combine per-core device outputs into the scalar loss."""
    const = hostctx['const']
    expend = hostctx['expend']
    L = hostctx['L']
    renorms = hostctx['renorms']
    nb = hostctx['nb']
    llh = np.zeros(nb, np.float64)
    for ci, res in enumerate(results):
        dv = np.asarray(res['devout'], np.float64)   # [T, nout]
        for b in range(bpc):
            bg = ci * bpc + b
            emisum = dv[:, b].sum()
            pfin = dv[:, bpc + b]                    # [T]
            pe = float(expend @ pfin)
            logs = 0.0
            for r, tr in enumerate(renorms):
                if tr < L[bg] - 1:
                    s_rb = dv[:, 2 * bpc + r * bpc + b].sum()
                    logs += np.log(s_rb)
            logZ = np.log(pe) + logs
            llh[bg] = const[bg] + emisum - logZ
    return np.float32(-llh.mean())


_PROGRAM_CACHE = {}


def kernel(**inputs):
    from concourse.bass_utils import run_bass_kernel_spmd
    in_maps, hostctx = host_prepare(inputs)
    key = (S, BPC, hostctx['renorms'], hostctx['inj_steps'])
    if key not in _PROGRAM_CACHE:
        _PROGRAM_CACHE[key] = build_program(
            s_len=S, bpc=BPC, renorms=hostctx['renorms'],
            inj_steps=hostctx['inj_steps'])
    nc, meta = _PROGRAM_CACHE[key]
    res = run_bass_kernel_spmd(nc, in_maps, core_ids=list(range(NCORES)))
    return host_finish(res.results, hostctx)

